# revision 18
# baseline (speedup 1.0000x reference)
"""CRPS loss kernel for Trainium2 (8 NeuronCores, SPMD).

Math: with |a-b| = 2*max(a,b) - a - b, for forecasts x_i (i<N) and obs y:
  T1 = sum_s sum_i |x_i - y|    = 2*Q - U - N*V
  T2 = sum_s sum_ij |x_i - x_j| = 4*Pm + (2-2N)*U
where
  Pm = sum_s sum_{i<j<N} max(x_i, x_j)   (device)
  Q  = sum_s sum_i max(x_i, y)           (device)
  U  = sum_s sum_i x_i,  V = sum_s y     (host, exact fp64 over fp16 inputs)
and crps_mean = T1/(N*S) - T2/(2*N^2*S).

max() is exact in fp16, so the only precision loss is fp16 input rounding
(measured rel err ~4e-7 vs the fp32 reference).

Device design (per core, spatial shard 65536 pts = [128 part, 512 free]):
- One SBUF tile holds all 20 members (member i at free cols [i*512,(i+1)*512));
  the Tile framework tracks sub-range deps, so pair segments that only read
  early members start while later member chunks are still streaming in.
- Pair (i, j=i+d) maxes are batched as contiguous diagonal-segment tensor_max
  ops, emitted in prefix-milestone order (members 0-2, 0-5, 0-9, all) to
  overlap the DMA; 1-block segments are emitted last to keep the tail short.
- Reduction of each 512-col max block runs on the otherwise-idle PE as a
  ones-vector matmul accumulating into PSUM. Pair sums split across two PSUM
  tiles so the first one drains (scalar-engine copy + DMA out) while the
  second still accumulates. Input DMAs use only the two HWDGE rings (sync /
  scalar): gpsimd SWDGE descriptor generation would deadlock against DVE
  2-port tensor_tensor ops (shared SBUF port lock).
"""

import numpy as np

N_CORES = 8
N = 20
S_FULL = 4 * 1 * 8 * 128 * 128  # 524288
S_LOC = S_FULL // N_CORES  # 65536
P = 128
F = S_LOC // P  # 512
MILESTONES = (3, 6, 10, 20)
PSUM_SPLIT = 100  # pair matmuls before this index accumulate into psum A

_CACHE = {}


def _segments():
    """Diagonal segments (i_start, d, n_blocks, milestone) emitted so that
    each group only reads members < its milestone. Within the final group,
    larger segments first (small ones keep the kernel tail short)."""
    groups = []
    prev = 0
    for m in MILESTONES:
        g = []
        for d in range(1, m):
            ilo = max(0, prev - d)
            ihi = m - 1 - d
            if ihi >= ilo:
                g.append((ilo, d, ihi - ilo + 1, m))
        groups.append(g)
        prev = m
    groups[-1].sort(key=lambda s: -s[2])
    return groups


def _build():
    import concourse.bacc as bacc
    import concourse.tile as tile
    import concourse.mybir as mybir

    f16 = mybir.dt.float16
    f32 = mybir.dt.float32

    nc = bacc.Bacc("TRN2", target_bir_lowering=False, debug=False, num_devices=N_CORES)
    # x is pre-transposed on host to [p, n, f] so DMA rows are contiguous
    x_d = nc.dram_tensor("x", [P, N * F], f16, kind="ExternalInput")
    y_d = nc.dram_tensor("y", [P, F], f16, kind="ExternalInput")
    out_d = nc.dram_tensor("out", [3, F], f32, kind="ExternalOutput")

    groups = _segments()
    n_pair_mm = sum(s[2] for g in groups for s in g)  # 190

    with tile.TileContext(nc) as tc:
        with (
            tc.tile_pool(name="data", bufs=1) as data,
            tc.tile_pool(name="scr", bufs=4) as scrp,
            tc.tile_pool(name="psum", bufs=1, space="PSUM") as pp,
        ):
            X = data.tile([P, N * F], f16)
            yt = data.tile([P, F], f16)
            ones = data.tile([P, 1], f16)
            outt = data.tile([1, 3 * F], f32)
            nc.vector.memset(ones[:], 1.0)

            xa = x_d.ap()
            # HWDGE rings only; first chunks smallest so compute starts early
            chunks = [(0, 3), (3, 6), (6, 10), (10, 15), (15, 20)]
            for ci, (lo, hi) in enumerate(chunks):
                eng = nc.sync if ci % 2 == 0 else nc.scalar
                eng.dma_start(out=X[:, lo * F : hi * F], in_=xa[:, lo * F : hi * F])
            nc.sync.dma_start(out=yt[:], in_=y_d.ap())

            psum_pa = pp.tile([1, F], f32)
            psum_pb = pp.tile([1, F], f32)
            psum_obs = pp.tile([1, F], f32)

            def obs_op(blk0, nblk, first, last):
                """max(x_i, y) for members blk0..blk0+nblk-1 -> psum_obs."""
                s = scrp.tile([P, M_SCR * F], f16, tag="scr")
                s3 = s[:].rearrange("p (n f) -> p n f", f=F)
                X3 = X[:].rearrange("p (n f) -> p n f", f=F)
                yb = yt[:].unsqueeze(1).broadcast_to([P, nblk, F])
                nc.vector.tensor_tensor(
                    s3[:, :nblk, :],
                    X3[:, blk0 : blk0 + nblk, :],
                    yb,
                    mybir.AluOpType.max,
                )
                for b in range(nblk):
                    nc.tensor.matmul(
                        psum_obs[:],
                        ones[:],
                        s[:, b * F : (b + 1) * F],
                        start=(first and b == 0),
                        stop=(last and b == nblk - 1),
                        skip_group_check=True,
                    )
                if last:
                    # obs psum complete mid-kernel: drain it while pairs run
                    nc.scalar.copy(out=outt[:, 2 * F :], in_=psum_obs[:])
                    nc.sync.dma_start(out=out_d[2:3, :], in_=outt[:, 2 * F :])

            M_SCR = 10
            kp = 0
            emitted_obs = 0

            def pair_seg(i0, d, nblk):
                nonlocal kp
                L = nblk * F
                s = scrp.tile([P, M_SCR * F], f16, tag="scr")
                nc.vector.tensor_max(
                    s[:, :L],
                    X[:, i0 * F : i0 * F + L],
                    X[:, (i0 + d) * F : (i0 + d) * F + L],
                )
                for b in range(nblk):
                    tgt = psum_pa if kp < PSUM_SPLIT else psum_pb
                    nc.tensor.matmul(
                        tgt[:],
                        ones[:],
                        s[:, b * F : (b + 1) * F],
                        start=(kp == 0 or kp == PSUM_SPLIT),
                        stop=(kp == PSUM_SPLIT - 1 or kp == n_pair_mm - 1),
                        skip_group_check=True,
                    )
                    kp += 1
                    if kp == PSUM_SPLIT:
                        # psum A complete: drain it while B accumulates
                        nc.scalar.copy(out=outt[:, :F], in_=psum_pa[:])
                        nc.sync.dma_start(out=out_d[0:1, :], in_=outt[:, :F])

            for gi, g in enumerate(groups):
                if gi == len(groups) - 1:
                    # t0 half loaded; fill the wait for late members with obs
                    obs_op(0, 10, first=True, last=False)
                    emitted_obs = 10
                    for si, seg in enumerate(g):
                        pair_seg(*seg[:3])
                        if si == 1:
                            obs_op(10, 10, first=False, last=True)
                else:
                    for seg in g:
                        pair_seg(*seg[:3])

            nc.scalar.copy(out=outt[:, F : 2 * F], in_=psum_pb[:])
            nc.sync.dma_start(out=out_d[1:2, :], in_=outt[:, F : 2 * F])

    nc.compile()
    return nc


def _get_nc():
    if "nc" not in _CACHE:
        _CACHE["nc"] = _build()
    return _CACHE["nc"]


def _shard_inputs(forecasts, observations):
    f = np.asarray(forecasts, dtype=np.float32).reshape(N, S_FULL).astype(np.float16)
    o = np.asarray(observations, dtype=np.float32).reshape(S_FULL).astype(np.float16)
    # device layout: [p, n, f] per core so each DMA row is contiguous
    fr = f.reshape(N, N_CORES, P, F)
    orr = o.reshape(N_CORES, P, F)
    in_maps = []
    for c in range(N_CORES):
        xc = np.ascontiguousarray(fr[:, c].transpose(1, 0, 2)).reshape(P, N * F)
        in_maps.append({"x": xc, "y": orr[c]})
    return f, o, in_maps


def _combine(f, o, outs):
    """outs: list of per-core [3, F] float32 arrays (pairsA, pairsB, obs)."""
    U = f.astype(np.float64).sum()
    V = o.astype(np.float64).sum()
    Pm = sum(out[0].astype(np.float64).sum() + out[1].astype(np.float64).sum()
             for out in outs)
    Q = sum(out[2].astype(np.float64).sum() for out in outs)
    T1 = 2.0 * Q - U - N * V
    T2 = 4.0 * Pm + (2.0 - 2.0 * N) * U
    crps = T1 / (N * S_FULL) - T2 / (2.0 * N * N * S_FULL)
    return np.float32(crps)


def kernel(forecasts, observations):
    from concourse.bass_utils import run_bass_kernel_spmd

    nc = _get_nc()
    f, o, in_maps = _shard_inputs(forecasts, observations)
    res = run_bass_kernel_spmd(nc, in_maps, list(range(N_CORES)))
    outs = [res.results[c]["out"] for c in range(N_CORES)]
    return _combine(f, o, outs)


# revision 25
# speedup vs baseline: 1.0127x; 1.0127x over previous
"""CRPS loss kernel for Trainium2 (8 NeuronCores, SPMD).

Math: with |a-b| = 2*max(a,b) - a - b, for forecasts x_i (i<N) and obs y:
  T1 = sum_s sum_i |x_i - y|    = 2*Q - U - N*V
  T2 = sum_s sum_ij |x_i - x_j| = 4*Pm + (2-2N)*U
where
  Pm = sum_s sum_{i<j<N} max(x_i, x_j)   (device)
  Q  = sum_s sum_i max(x_i, y)           (device)
  U  = sum_s sum_i x_i,  V = sum_s y     (host, exact fp64 over fp16 inputs)
and crps_mean = T1/(N*S) - T2/(2*N^2*S).

max() is exact in fp16, so the only precision loss is fp16 input rounding
(measured rel err ~4e-7 vs the fp32 reference).

Device design (per core, spatial shard 65536 pts = [128 part, 512 free]):
- One SBUF tile holds all 20 members (member i at free cols [i*512,(i+1)*512));
  the Tile framework tracks sub-range deps, so pair segments that only read
  early members start while later member chunks are still streaming in.
- Pair (i, j=i+d) maxes are batched as contiguous diagonal-segment tensor_max
  ops, emitted in prefix-milestone order (members 0-2, 0-5, 0-9, all) to
  overlap the DMA; 1-block segments are emitted last to keep the tail short.
- Reduction of each 512-col max block runs on the otherwise-idle PE as a
  ones-vector matmul accumulating into PSUM. Pair sums split across two PSUM
  tiles so the first one drains (scalar-engine copy + DMA out) while the
  second still accumulates. Input DMAs use only the two HWDGE rings (sync /
  scalar): gpsimd SWDGE descriptor generation would deadlock against DVE
  2-port tensor_tensor ops (shared SBUF port lock).
"""

import numpy as np

N_CORES = 8
N = 20
S_FULL = 4 * 1 * 8 * 128 * 128  # 524288
S_LOC = S_FULL // N_CORES  # 65536
P = 128
F = S_LOC // P  # 512
MILESTONES = (3, 6, 10, 20)
PSUM_SPLIT = 75  # PE pair matmuls before this index accumulate into psum A
N_ACT_SEGS = 4  # big final-group segments reduced on the scalar engine

_CACHE = {}


def _segments():
    """Diagonal segments (i_start, d, n_blocks, milestone) emitted so that
    each group only reads members < its milestone. Within the final group,
    larger segments first (small ones keep the kernel tail short)."""
    groups = []
    prev = 0
    for m in MILESTONES:
        g = []
        for d in range(1, m):
            ilo = max(0, prev - d)
            ihi = m - 1 - d
            if ihi >= ilo:
                g.append((ilo, d, ihi - ilo + 1, m))
        groups.append(g)
        prev = m
    groups[-1].sort(key=lambda s: -s[2])
    return groups


def _build():
    import concourse.bacc as bacc
    import concourse.tile as tile
    import concourse.mybir as mybir

    f16 = mybir.dt.float16
    f32 = mybir.dt.float32

    nc = bacc.Bacc("TRN2", target_bir_lowering=False, debug=False, num_devices=N_CORES)
    # x is pre-transposed on host to [p, n, f] so DMA rows are contiguous
    x_d = nc.dram_tensor("x", [P, N * F], f16, kind="ExternalInput")
    y_d = nc.dram_tensor("y", [P, F], f16, kind="ExternalInput")
    out_d = nc.dram_tensor("out", [3, F], f32, kind="ExternalOutput")
    out2_d = nc.dram_tensor("out2", [P, N_ACT_SEGS], f32, kind="ExternalOutput")

    groups = _segments()
    # blocks reduced by PE matmuls (ACT-routed segments excluded)
    n_pair_mm = sum(
        s[2]
        for gi, g in enumerate(groups)
        for si, s in enumerate(g)
        if not (gi == len(groups) - 1 and si < N_ACT_SEGS)
    )

    with tile.TileContext(nc) as tc:
        with (
            tc.tile_pool(name="data", bufs=1) as data,
            tc.tile_pool(name="scr", bufs=4) as scrp,
            tc.tile_pool(name="psum", bufs=1, space="PSUM") as pp,
        ):
            X = data.tile([P, N * F], f16)
            yt = data.tile([P, F], f16)
            ones = data.tile([P, 1], f16)
            outt = data.tile([1, 3 * F], f32)
            nc.vector.memset(ones[:], 1.0)

            xa = x_d.ap()
            # HWDGE rings only; first chunks smallest so compute starts early
            chunks = [(0, 3), (3, 6), (6, 10), (10, 15), (15, 20)]
            for ci, (lo, hi) in enumerate(chunks):
                eng = nc.sync if ci % 2 == 0 else nc.scalar
                eng.dma_start(out=X[:, lo * F : hi * F], in_=xa[:, lo * F : hi * F])
            nc.sync.dma_start(out=yt[:], in_=y_d.ap())

            psum_pa = pp.tile([1, F], f32)
            psum_pb = pp.tile([1, F], f32)
            psum_obs = pp.tile([1, F], f32)

            def obs_op(blk0, nblk, first, last):
                """max(x_i, y) for members blk0..blk0+nblk-1 -> psum_obs."""
                s = scrp.tile([P, M_SCR * F], f16, tag="scr")
                s3 = s[:].rearrange("p (n f) -> p n f", f=F)
                X3 = X[:].rearrange("p (n f) -> p n f", f=F)
                yb = yt[:].unsqueeze(1).broadcast_to([P, nblk, F])
                nc.vector.tensor_tensor(
                    s3[:, :nblk, :],
                    X3[:, blk0 : blk0 + nblk, :],
                    yb,
                    mybir.AluOpType.max,
                )
                for b in range(nblk):
                    nc.tensor.matmul(
                        psum_obs[:],
                        ones[:],
                        s[:, b * F : (b + 1) * F],
                        start=(first and b == 0),
                        stop=(last and b == nblk - 1),
                        skip_group_check=True,
                    )
                if last:
                    # obs psum complete mid-kernel: drain it while pairs run
                    nc.scalar.copy(out=outt[:, 2 * F :], in_=psum_obs[:])
                    nc.sync.dma_start(out=out_d[2:3, :], in_=outt[:, 2 * F :])

            M_SCR = 10
            kp = 0
            emitted_obs = 0

            acc_act = data.tile([P, N_ACT_SEGS], f32)
            n_act = 0

            def pair_seg(i0, d, nblk, act_reduce=False):
                nonlocal kp, n_act
                L = nblk * F
                s = scrp.tile([P, M_SCR * F], f16, tag="scr")
                nc.vector.tensor_max(
                    s[:, :L],
                    X[:, i0 * F : i0 * F + L],
                    X[:, (i0 + d) * F : (i0 + d) * F + L],
                )
                if act_reduce:
                    # per-partition sum on the mostly-idle scalar engine,
                    # freeing the PE (which otherwise runs even with DVE)
                    ascr = scrp.tile([P, M_SCR * F], f16, tag="ascr")
                    nc.scalar.activation(
                        out=ascr[:, :L],
                        in_=s[:, :L],
                        func=mybir.ActivationFunctionType.Copy,
                        accum_out=acc_act[:, n_act : n_act + 1],
                    )
                    n_act += 1
                    return
                for b in range(nblk):
                    tgt = psum_pa if kp < PSUM_SPLIT else psum_pb
                    nc.tensor.matmul(
                        tgt[:],
                        ones[:],
                        s[:, b * F : (b + 1) * F],
                        start=(kp == 0 or kp == PSUM_SPLIT),
                        stop=(kp == PSUM_SPLIT - 1 or kp == n_pair_mm - 1),
                        skip_group_check=True,
                    )
                    kp += 1
                    if kp == PSUM_SPLIT:
                        # psum A complete: drain it while B accumulates
                        nc.scalar.copy(out=outt[:, :F], in_=psum_pa[:])
                        nc.sync.dma_start(out=out_d[0:1, :], in_=outt[:, :F])

            for gi, g in enumerate(groups):
                if gi == len(groups) - 1:
                    # t0 half loaded; fill the wait for late members with obs
                    obs_op(0, 10, first=True, last=False)
                    emitted_obs = 10
                    for si, seg in enumerate(g):
                        pair_seg(*seg[:3], act_reduce=(si < N_ACT_SEGS))
                        if si == 1:
                            obs_op(10, 10, first=False, last=True)
                else:
                    for seg in g:
                        pair_seg(*seg[:3])

            nc.scalar.dma_start(out=out2_d.ap(), in_=acc_act[:])
            nc.scalar.copy(out=outt[:, F : 2 * F], in_=psum_pb[:])
            nc.sync.dma_start(out=out_d[1:2, :], in_=outt[:, F : 2 * F])

    nc.compile()
    return nc


def _get_nc():
    if "nc" not in _CACHE:
        _CACHE["nc"] = _build()
    return _CACHE["nc"]


def _shard_inputs(forecasts, observations):
    f = np.asarray(forecasts, dtype=np.float32).reshape(N, S_FULL).astype(np.float16)
    o = np.asarray(observations, dtype=np.float32).reshape(S_FULL).astype(np.float16)
    # device layout: [p, n, f] per core so each DMA row is contiguous
    fr = f.reshape(N, N_CORES, P, F)
    orr = o.reshape(N_CORES, P, F)
    in_maps = []
    for c in range(N_CORES):
        xc = np.ascontiguousarray(fr[:, c].transpose(1, 0, 2)).reshape(P, N * F)
        in_maps.append({"x": xc, "y": orr[c]})
    return f, o, in_maps


def _combine(f, o, outs, outs2):
    """outs: per-core [3, F] (pairsA, pairsB, obs); outs2: per-core [P, N_ACT_SEGS]
    scalar-engine pair partials."""
    U = f.astype(np.float64).sum()
    V = o.astype(np.float64).sum()
    Pm = sum(out[0].astype(np.float64).sum() + out[1].astype(np.float64).sum()
             for out in outs)
    Pm += sum(o2.astype(np.float64).sum() for o2 in outs2)
    Q = sum(out[2].astype(np.float64).sum() for out in outs)
    T1 = 2.0 * Q - U - N * V
    T2 = 4.0 * Pm + (2.0 - 2.0 * N) * U
    crps = T1 / (N * S_FULL) - T2 / (2.0 * N * N * S_FULL)
    return np.float32(crps)


def kernel(forecasts, observations):
    from concourse.bass_utils import run_bass_kernel_spmd

    nc = _get_nc()
    f, o, in_maps = _shard_inputs(forecasts, observations)
    res = run_bass_kernel_spmd(nc, in_maps, list(range(N_CORES)))
    outs = [res.results[c]["out"] for c in range(N_CORES)]
    outs2 = [res.results[c]["out2"] for c in range(N_CORES)]
    return _combine(f, o, outs, outs2)
